# revision 84
# baseline (speedup 1.0000x reference)
"""Node2GraphAttention Trainium2 kernel (8-core SPMD).

Computes, for sorted segment ids n_batch over N nodes:
    coefs = sigmoid(sum(n_embedding * g_embedding[n_batch], axis=1))
    out   = segment_sum(coefs[:, None] * n_embedding, n_batch, G)

Strategy: shard nodes across 8 cores at graph boundaries (each graph fully on
one core -> no cross-core reduction). Per core, graphs are packed into blocks
of <=128 graph slots; nodes stream in 512-node super-tiles. Sortedness lets
both the gather and the scatter be expressed as matmuls against masks built
with single DVE tensor_scalar ops:
  - gather: U[j,i] = (i >= start_of_graph_j), gathered = U.T @ dG where dG is
    the per-block first-difference of g rows (prefix sums telescope to g[idx]).
  - dot:    DVE multiply + free-axis reduce -> s columns, sigmoid on ACT.
  - scatter: mask[i,j] = (idx_i == j) * coef_i via one two-op tensor_scalar
    (split between DVE and Pool engines); PSUM accumulates mask.T @ n over the
    block and is DMA'd straight to HBM.
Node embeddings are DMA'd once per block (1 descriptor) to keep the SP
sequencer (~700ns/DMA issue) off the critical path. The per-super work is
software-pipelined over 5 stages (U+gather / PSUM evac / dot / sigmoid /
mask+scatter) so the cross-engine hops overlap across supers; the gather uses
K=32 matmuls against per-super dg rows packed 3 supers per 128-partition tile
(matmul base partitions 0/32/64).
"""

import sys

if "/opt/trn_rl_repo" not in sys.path:
    sys.path.insert(0, "/opt/trn_rl_repo")

import numpy as np

import concourse.bacc as bacc
import concourse.mybir as mybir
import concourse.tile as tile
from concourse.bass_utils import run_bass_kernel_spmd

N_CORES = 8
D = 128          # embedding dim
GS = 128         # graph slots per block
SUP = 512        # nodes per super-tile
SUBT = SUP // 128
CAP_NODES = 12 * SUP  # max nodes per block (greedy packing target)
RMAX = 32        # dg rows per super (base row + <=31 graph starts)
UPACK = 3        # supers per packed-U build (matmul base partition: 0/32/64)
RPG = 4          # row-groups of RMAX per 128-partition pack tile

FP16 = mybir.dt.float16
F32 = mybir.dt.float32

# tuning knobs (read at program-build time; part of the cache key)
CFG = {
    "act_red": 0,     # subtiles whose d-reduce rides the ACT accumulator
    "stt": True,      # fused scalar_tensor_tensor multiply+row-sum dot
    "b0_chunks": (6, 6),  # block-0 node-DMA startup split
    "mask_dve": 1,    # how many of the 4 mask tensor_scalar ops go to DVE
    "mask_last": False,  # DVE mask on last subtile instead of first
    "mask_duty": (3, 4),  # DVE takes its mask(s) on num-of-den supers
    "nchunk": 4,      # supers per node-embedding DMA chunk
    "b2slot": 4,      # pipeline slot of the dot stage
    "sigb": 1,        # supers per batched sigmoid
    "skew": 7,        # software-pipeline distance between gather and scatter
    "ramp": 0,        # compress stage offsets at pipeline fill/drain
    "u_pool": False,  # build U on gpsimd instead of vector
}


# ---------------------------------------------------------------- host planning

def _core_graph_cuts(boundaries, n_cores):
    """Split graphs into n_cores contiguous ranges with ~equal node counts."""
    G = len(boundaries) - 1
    N = int(boundaries[-1])
    cuts = [0]
    for m in range(1, n_cores):
        target = (N * m) // n_cores
        g = int(np.searchsorted(boundaries, target))
        if g > 0 and (target - boundaries[g - 1]) < (boundaries[g] - target if g <= G else 10**18):
            g = g - 1
        g = min(max(g, cuts[-1]), G)
        cuts.append(g)
    cuts.append(G)
    return cuts


def _pack_blocks(boundaries, glo, ghi):
    """Greedy: blocks of <=GS graphs and (if possible) <=CAP_NODES nodes."""
    blocks = []
    g = glo
    while g < ghi:
        g2 = min(g + GS, ghi)
        # shrink until node count fits (keep at least one graph)
        while g2 > g + 1 and boundaries[g2] - boundaries[g] > CAP_NODES:
            g2 = g + int(np.searchsorted(
                boundaries[g + 1:g2 + 1], boundaries[g] + CAP_NODES, side="right"))
            g2 = max(g2, g + 1)
            if boundaries[g2] - boundaries[g] > CAP_NODES and g2 > g + 1:
                g2 -= 1
            break
        while g2 > g + 1 and boundaries[g2] - boundaries[g] > CAP_NODES:
            g2 -= 1
        blocks.append((int(g), int(g2)))
        g = g2
    return blocks


def _pack_supers(boundaries, glo, ghi):
    """Split a block's node range into supers of <=SUP nodes covering at most
    RMAX-1 graph starts each. Returns list of (n0, n1, j0) with j0 = graph
    containing node n0 (block-relative graph indices are j - glo)."""
    nlo, nhi = int(boundaries[glo]), int(boundaries[ghi])
    supers = []
    n0 = nlo
    j0 = glo
    while n0 < nhi:
        n1 = min(n0 + SUP, nhi)
        # graph starts strictly inside (n0, n1): boundaries of graphs
        # j0+1..ghi-1 that fall before n1; cap their count at RMAX-1
        hi_start = int(np.searchsorted(boundaries[j0 + 1:ghi], n1,
                                       side="left"))
        if hi_start > RMAX - 1:
            n1 = int(boundaries[j0 + RMAX])
        supers.append((int(n0), int(n1), int(j0)))
        n0 = n1
        j0 = min(int(np.searchsorted(boundaries, n1, side="right")) - 1,
                 ghi - 1)
    return supers


def _plan(n_batch, G):
    N = len(n_batch)
    boundaries = np.searchsorted(n_batch, np.arange(G + 1))
    cuts = _core_graph_cuts(boundaries, N_CORES)
    core_blocks = [
        _pack_blocks(boundaries, cuts[c], cuts[c + 1]) for c in range(N_CORES)
    ]
    B = max(len(b) for b in core_blocks)
    core_supers = []  # [core][block] -> list of (n0, n1, j0)
    for c in range(N_CORES):
        per_block = []
        for b in range(B):
            if b < len(core_blocks[c]):
                glo, ghi = core_blocks[c][b]
                per_block.append(_pack_supers(boundaries, glo, ghi))
            else:
                per_block.append([])
        core_supers.append(per_block)
    S = []  # supers per block position (max over cores)
    for b in range(B):
        S.append(max(1, max(len(core_supers[c][b]) for c in range(N_CORES))))
    return boundaries, cuts, core_blocks, core_supers, B, S


# ---------------------------------------------------------------- device program

_PROGRAM_CACHE = {}


def _build_program(B, S, kloop=0):
    """Build the SPMD bass program for B blocks with S[b] super-tiles each.

    kloop > 0 wraps the whole body in a For_i repeat loop (timing rig only).
    """
    key = (B, tuple(S), kloop, tuple(sorted(CFG.items())))
    if key in _PROGRAM_CACHE:
        return _PROGRAM_CACHE[key]

    S_total = sum(S)
    nc = bacc.Bacc("TRN2", target_bir_lowering=False, debug=False,
                   num_devices=N_CORES)

    NG = (S_total + UPACK - 1) // UPACK
    n_in = nc.dram_tensor("n_in", [S_total, 128, SUBT, D], FP16,
                          kind="ExternalInput").ap()
    aux_in = nc.dram_tensor("aux_in", [S_total, 128, SUBT + 1], F32,
                            kind="ExternalInput").ap()
    pack_in = nc.dram_tensor("pack_in", [NG, 128, D], FP16,
                             kind="ExternalInput").ap()
    iota_in = nc.dram_tensor("iota_in", [128, SUP], FP16,
                             kind="ExternalInput").ap()
    out_dram = nc.dram_tensor("out", [B * GS, D], F32,
                              kind="ExternalOutput").ap()

    with tile.TileContext(nc) as tc:
        with (
            tc.tile_pool(name="singles", bufs=1) as singles,
            tc.tile_pool(name="npool", bufs=3) as npool,
            tc.tile_pool(name="upool", bufs=4) as upool,
            tc.tile_pool(name="gpool", bufs=9) as gpool,
            tc.tile_pool(name="mpool", bufs=3) as mpool,
            tc.tile_pool(name="scrp", bufs=5) as scrp,
            tc.tile_pool(name="coefp", bufs=4) as coefp,
            tc.tile_pool(name="auxp", bufs=2) as auxp,
            tc.tile_pool(name="dgp", bufs=8) as dgp,
            tc.tile_pool(name="ps_g", bufs=6, space="PSUM") as ps_g,
            tc.tile_pool(name="ps_o", bufs=2, space="PSUM") as ps_o,
        ):
            iota = singles.tile([128, SUP], FP16)
            nc.sync.dma_start(out=iota, in_=iota_in)

            import contextlib
            loop_cm = tc.For_i(0, kloop, 1) if kloop else contextlib.nullcontext()
            with loop_cm:
                _build_body(nc, tc, B, S, iota, n_in, aux_in, pack_in,
                            out_dram,
                            npool, upool, gpool, mpool, scrp, coefp, auxp, dgp,
                            ps_g, ps_o)

    nc.compile()
    _PROGRAM_CACHE[key] = nc
    return nc


def _build_body(nc, tc, B, S, iota, n_in, aux_in, pack_in,
                out_dram,
                npool, upool, gpool, mpool, scrp, coefp, auxp, dgp,
                ps_g, ps_o):
    SIGB = CFG["sigb"]
    SKEW = CFG["skew"]

    # global flattened super schedule: task i = (block b, local super s)
    tasks = [(b, s) for b in range(B) for s in range(S[b])]
    n_tasks = len(tasks)
    block_off = np.cumsum([0] + list(S))  # global super offset per block

    NCHUNK = CFG["nchunk"]
    blk_tiles = {}   # b -> (aux_sb, psum_out)
    chk_tiles = {}   # (b, s//NCHUNK) -> n_chunk tile
    grp_tiles = {}   # i//UPACK -> (u_pack, dgp_sb)
    state = {}       # per-task carried tiles

    def ensure_block(b):
        if b in blk_tiles:
            return blk_tiles[b]
        nsup = S[b]
        s0 = int(block_off[b])
        aux_sb = auxp.tile([128, nsup, SUBT + 1], F32)
        nc.sync.dma_start(
            out=aux_sb,
            in_=aux_in[s0:s0 + nsup].rearrange("s p c -> p s c"))
        psum_out = ps_o.tile([GS, D], F32)
        blk_tiles[b] = (aux_sb, psum_out)
        return blk_tiles[b]

    # block 0 loads in graduated pieces so data lands just ahead of the
    # warming pipeline; later blocks are fully prefetched anyway
    b0cuts = [0]
    for w in CFG["b0_chunks"]:
        if b0cuts[-1] < S[0]:
            b0cuts.append(min(b0cuts[-1] + w, S[0]))
    while b0cuts[-1] < S[0]:
        b0cuts.append(S[0])

    def b0_ck(s):
        return next(i for i in range(len(b0cuts) - 1) if s < b0cuts[i + 1])

    def ensure_chunk(b, s):
        ck = b0_ck(s) if b == 0 else 0
        key = (b, ck)
        if key in chk_tiles:
            return chk_tiles[key]
        if b == 0:
            c0, c1 = b0cuts[ck], b0cuts[ck + 1]
        else:
            c0, c1 = 0, S[b]
        s0 = int(block_off[b]) + c0
        n_chk = npool.tile([128, c1 - c0, SUBT, D], FP16)
        nc.sync.dma_start(
            out=n_chk,
            in_=n_in[s0:s0 + c1 - c0].rearrange("s p t d -> p s t d"))
        chk_tiles[key] = n_chk
        if b == 0 and c1 < S[0]:
            ensure_chunk(0, c1)  # queue the rest right behind, in order
        return n_chk

    def chunk_col(b, s):
        if b == 0:
            return s - b0cuts[b0_ck(s)]
        return s

    NGRP = (n_tasks + UPACK - 1) // UPACK
    grp_dma = {}

    PKB = 4  # U-groups per pack DMA batch

    def ensure_grp_dma(g):
        if g >= NGRP:
            return
        p = g // PKB
        if p in grp_dma:
            return
        p0 = p * PKB
        pw = min(PKB, NGRP - p0)
        pk_sb = dgp.tile([128, pw, D], FP16)
        nc.sync.dma_start(
            out=pk_sb,
            in_=pack_in[p0:p0 + pw].rearrange("g p d -> p g d"))
        grp_dma[p] = pk_sb

    def stage_a(i):
        b, s = tasks[i]
        g = i // UPACK
        r0 = (i % UPACK) * RMAX
        aux_sb, _ = ensure_block(b)
        if i % UPACK == 0:
            ensure_grp_dma(g)
            ensure_grp_dma(g + PKB)  # next batch ahead of big n DMAs
            dgp_sb = grp_dma[g // PKB][:, g % PKB, :]
            u_pack = upool.tile([128, SUP], FP16)
            u_eng = nc.gpsimd if CFG["u_pool"] else nc.vector
            u_eng.tensor_scalar(
                out=u_pack, in0=iota,
                scalar1=aux_sb[:, s, SUBT:SUBT + 1], scalar2=None,
                op0=mybir.AluOpType.is_ge,
            )
            grp_tiles[g] = (u_pack, dgp_sb)
        ensure_chunk(b, s)
        u_pack, dgp_sb = grp_tiles[g]
        gath_ps = ps_g.tile([128, SUBT, D], F32)
        for t in range(SUBT):
            nc.tensor.matmul(
                gath_ps[:, t, :],
                lhsT=u_pack[r0:r0 + RMAX, t * 128:(t + 1) * 128],
                rhs=dgp_sb[r0:r0 + RMAX, :],
                start=True, stop=True,
            )
        state[i] = {"gath_ps": gath_ps}

    def stage_b1(i):
        # PSUM -> SBUF evacuation of the gathered g rows
        st = state[i]
        gath_sb = gpool.tile([128, 1, SUBT, D], FP16)
        nc.scalar.copy(gath_sb, st.pop("gath_ps"))
        st["gath_sb"] = (gath_sb, 0)

    def stage_b2(i):
        # multiply + reduce -> s columns of the group coef tile
        b, s = tasks[i]
        n_chk = ensure_chunk(b, s)
        sc = chunk_col(b, s)
        st = state[i]
        gath_sb, ghalf = st.pop("gath_sb")
        gi = i % SIGB   # slot within sigmoid group
        if gi == 0:
            coef_in = scrp.tile([128, min(SIGB, n_tasks - i) * SUBT], F32)
            st["coef_in"] = coef_in
        else:
            coef_in = state[i - gi]["coef_in"]
        if CFG["stt"]:
            # fused multiply+accumulate per subtile: one TensorScalarPtr in
            # scalar_tensor_tensor mode computes (n*1)*gath and its row-sum
            prod = gpool.tile([128, SUBT, D], FP16)
            for t in range(SUBT):
                nc.vector.scalar_tensor_tensor(
                    out=prod[:, t], in0=n_chk[:, sc, t], scalar=1.0,
                    in1=gath_sb[:, ghalf, t],
                    op0=mybir.AluOpType.mult, op1=mybir.AluOpType.mult,
                    accum_out=coef_in[:, gi * SUBT + t:gi * SUBT + t + 1])
            return
        prod = gpool.tile([128, SUBT, D], FP16)
        nc.vector.tensor_mul(prod, n_chk[:, sc], gath_sb[:, ghalf])
        # reduce: first SUBT-act_red subtiles on DVE here; the rest ride the
        # ACT accumulator one pipeline slot later (stage_b3)
        ar = CFG["act_red"]
        kd = SUBT - ar
        if kd:
            nc.vector.reduce_sum(
                coef_in[:, gi * SUBT:gi * SUBT + kd], prod[:, :kd],
                axis=mybir.AxisListType.X)
        if ar:
            st["prod"] = prod

    def stage_b3(i):
        # ACT-accumulator reduce for the remaining subtiles
        ar = CFG["act_red"]
        if not ar:
            return
        st = state[i]
        prod = st.pop("prod")
        gi = i % SIGB
        coef_in = state[i - gi]["coef_in"]
        for t in range(SUBT - ar, SUBT):
            ascr = gpool.tile([128, D], FP16)
            nc.scalar.activation(
                ascr, prod[:, t], mybir.ActivationFunctionType.Copy,
                accum_out=coef_in[:, gi * SUBT + t:gi * SUBT + t + 1])

    def stage_c(i):
        # sigmoid for the group ending at i
        gi = i % SIGB
        if gi == SIGB - 1 or i == n_tasks - 1:
            g0 = i - gi
            coef_in = state[g0]["coef_in"]
            coef = coefp.tile([128, coef_in.shape[-1]], F32)
            nc.scalar.activation(
                coef, coef_in, mybir.ActivationFunctionType.Sigmoid)
            for j in range(g0, i + 1):
                state[j]["coef"] = (coef, (j - g0) * SUBT)

    def stage_d(i):
        b, s = tasks[i]
        aux_sb, psum_out = ensure_block(b)
        n_chk = ensure_chunk(b, s)
        sc = chunk_col(b, s)
        st = state.pop(i)
        coef, co = st["coef"]
        nsup = S[b]
        mask = mpool.tile([128, SUBT, GS], FP16)
        num, den = CFG["mask_duty"]
        n_dve = CFG["mask_dve"] if (i * num) % den < num else 0
        for t in range(SUBT):
            on_dve = (t >= SUBT - n_dve) if CFG["mask_last"] else (t < n_dve)
            eng = nc.vector if on_dve else nc.gpsimd
            eng.tensor_scalar(
                out=mask[:, t, :], in0=iota[:, :GS],
                scalar1=aux_sb[:, s, t:t + 1],
                scalar2=coef[:, co + t:co + t + 1],
                op0=mybir.AluOpType.is_equal,
                op1=mybir.AluOpType.mult,
            )
        for t in range(SUBT):
            nc.tensor.matmul(
                psum_out,
                lhsT=mask[:, t, :],
                rhs=n_chk[:, sc, t, :],
                start=(s == 0 and t == 0),
                stop=(s == nsup - 1 and t == SUBT - 1),
            )
        if s == nsup - 1:
            out_sb = coefp.tile([GS, D], F32)
            nc.scalar.copy(out_sb, psum_out)
            nc.sync.dma_start(out=out_dram[b * GS:(b + 1) * GS, :],
                              in_=out_sb)
            del blk_tiles[b]

    B2S = CFG["b2slot"]
    CS = B2S + (2 if CFG["act_red"] else 1)  # sigmoid slot
    RAMP = CFG["ramp"]

    def off(j, steady, floor):
        # stage offset for task j: compressed while the pipeline fills or
        # drains (engines idle there anyway), steady-state skew in between
        if not RAMP:
            return steady
        if RAMP == 2:  # tail-only compression
            return min(steady, floor + (n_tasks - 1 - j))
        return min(steady, floor + j, floor + (n_tasks - 1 - j))

    # per-stage emission schedules (iteration -> tasks); offsets are
    # monotone in j so per-stage task order is preserved
    sched = {}
    for j in range(n_tasks):
        sched.setdefault(j, []).append(("a", j))
        sched.setdefault(j + 1, []).append(("b1", j))
        sched.setdefault(j + off(j, B2S, 2), []).append(("b2", j))
        sched.setdefault(j + off(j, B2S, 2) + 1, []).append(("b3", j))
        sched.setdefault(j + off(j, CS, 3), []).append(("c", j))
        sched.setdefault(j + off(j, SKEW, 4), []).append(("d", j))
    fns = {"a": stage_a, "b1": stage_b1, "b2": stage_b2, "b3": stage_b3,
           "c": stage_c, "d": stage_d}
    order = {"a": 0, "b1": 1, "b2": 2, "b3": 3, "c": 4, "d": 5}
    for i in sorted(sched):
        for st, j in sorted(sched[i], key=lambda x: (order[x[0]], x[1])):
            fns[st](j)


# ---------------------------------------------------------------- host assembly

def _assemble_core(n_embedding, g_embedding, boundaries, blocks, supers_pb,
                   B, S):
    """Build one core's padded input arrays."""
    S_total = sum(S)
    NG = (S_total + UPACK - 1) // UPACK
    n_arr = np.zeros((S_total, 128, SUBT, D), np.float16)
    aux_arr = np.zeros((S_total, 128, SUBT + 1), np.float32)
    pack = np.zeros((NG, 128, D), np.float16)
    apack = np.full((NG, 128), 1024.0, np.float32)
    apack_flat = apack.reshape(NG * RPG, RMAX)
    dgpack_flat = pack.reshape(NG * RPG, RMAX, D)
    # base rows always active (incl. padding supers: dg row 0 stays zero)
    apack_flat[:, 0] = 0.0

    s_base = 0
    for b in range(B):
        nsup = S[b]
        if b < len(blocks):
            glo, ghi = blocks[b]
            supers = supers_pb[b]
            for si, (n0, n1, j0) in enumerate(supers):
                gs = s_base + si
                fi = (gs // UPACK) * RPG + gs % UPACK  # row-group index
                nn = n1 - n0
                # node embeddings, [p][t][d] with local node = t*128 + p
                nblk = np.zeros((SUP, D), np.float16)
                nblk[:nn] = n_embedding[n0:n1].astype(np.float16)
                n_arr[gs] = nblk.reshape(SUBT, 128, D).transpose(1, 0, 2)

                # per-node graph slot rel. to block (pad nodes -> last slot)
                idx = np.full(SUP, ghi - glo - 1, np.int64)
                idx[:nn] = np.searchsorted(
                    boundaries, np.arange(n0, n1), side="right") - 1 - glo
                aux_arr[gs, :, :SUBT] = (
                    idx.reshape(SUBT, 128).transpose(1, 0)
                    .astype(np.float32))

                # dg rows: base = g[j0], then diffs at graph starts inside
                dgpack_flat[fi, 0] = g_embedding[j0].astype(np.float16)
                r = 1
                for j in range(j0 + 1, ghi):
                    a = int(boundaries[j]) - n0
                    if a >= nn:
                        break
                    apack_flat[fi, r] = float(a)
                    dgpack_flat[fi, r] = (
                        g_embedding[j].astype(np.float32)
                        - g_embedding[j - 1].astype(np.float32)
                    ).astype(np.float16)
                    r += 1
                    if r >= RMAX:
                        break
        s_base += nsup

    # each group's a-column rides the aux row of its first super
    for g in range(NG):
        gs = g * UPACK
        if gs < S_total:
            aux_arr[gs, :, SUBT] = apack[g]
    return {"n_in": n_arr, "aux_in": aux_arr, "pack_in": pack}


def _make_in_maps(n_embedding, g_embedding, n_batch, G, plan):
    boundaries, cuts, core_blocks, core_supers, B, S = plan
    iota = np.broadcast_to(
        np.arange(SUP, dtype=np.float16)[None, :], (128, SUP)).copy()
    in_maps = []
    for c in range(N_CORES):
        m = _assemble_core(n_embedding, g_embedding, boundaries,
                           core_blocks[c], core_supers[c], B, S)
        m["iota_in"] = iota
        in_maps.append(m)
    return in_maps


def _unshard(results, plan, G):
    boundaries, cuts, core_blocks, core_supers, B, S = plan
    out = np.zeros((G, D), np.float32)
    for c in range(N_CORES):
        res = results[c]["out"]
        for b, (glo, ghi) in enumerate(core_blocks[c]):
            out[glo:ghi] = res[b * GS:b * GS + (ghi - glo)]
    return out


# ---------------------------------------------------------------- entry point

def kernel(n_embedding, g_embedding, n_batch, size):
    n_embedding = np.asarray(n_embedding, dtype=np.float32)
    g_embedding = np.asarray(g_embedding, dtype=np.float32)
    n_batch = np.asarray(n_batch)
    G = int(size)

    plan = _plan(n_batch, G)
    B, S = plan[-2], plan[-1]
    nc = _build_program(B, S)
    in_maps = _make_in_maps(n_embedding, g_embedding, n_batch, G, plan)
    res = run_bass_kernel_spmd(nc, in_maps, core_ids=list(range(N_CORES)))
    return _unshard(res.results, plan, G)
